# revision 16
# baseline (speedup 1.0000x reference)
"""Trainium2 Bass kernel for nn_ContextAddition (ragged sequence insertion).

Math: for each row b with first-EOT position e = argmin{p: tok[b,p]==EOT} and
shift = 16 if dynamic_bools[b] else 8, the reference output reduces to a pure
row-gather from an extended embedding table T = [token_embedding; da; ca]:

    out[b,p] = T[ tok[b,p] ]            if p <  e
             = T[ VOCAB + (p - e) ]     if e <= p < e + shift   (da rows then ca rows)
             = T[ tok[b, p - shift] ]   if p >= e + shift

(The da insertion applies to all rows; the ca insertion only to dynamic rows,
and since da precedes ca in T, VOCAB + (p - e) indexes both uniformly.)

Design (MODE "hostidx"): the host computes the full int32 index map [B,77]
(cheap numpy) and uploads it per-core as [128, NT*77]; the device does one
[128,1]-offset indirect DMA per position (the only indirect form the SWDGE
ucode walks correctly: one descriptor per partition) and plain HWDGE writes.
The SWDGE descriptor-gen fixed cost (~994ns/instruction, serialized on the
Pool engine) is the structural bottleneck at 154 instructions/core, so the
gathers are round-robined across NSWQ SWDGE rings (qPoolDynamic{,1,2,3}) to
parallelize descriptor generation across Q7 cores. Table and output travel
as f16 (host upcasts output; rel err ~2e-4).

Device compute of the index map (MODE "indirect") kept as fallback.

Pure data parallel over 8 cores (256 batch rows each).
"""

import sys

import numpy as np

from concourse import bacc, bass, mybir
import concourse.tile as tile
from concourse.bass_utils import run_bass_kernel_spmd


def _ensure_profiling_hooks():
    """Make NTFF tracing under axon non-fatal / functional if BASS_TRACE is
    set by the caller: register the antenv.axon_hooks shim when the real
    module is absent, and make artifact upload failures non-fatal."""
    try:
        import antenv.axon_hooks  # noqa: F401
    except ImportError:
        try:
            import contextlib as _cl
            import types as _t

            import antenv
            from trn_agent_boot.trn_boot import _ntff_profile_via_ctypes

            hook = _ntff_profile_via_ctypes("/opt/axon/libaxon_pjrt.so")

            if hook is not None:
                _raw = hook

                @_cl.contextmanager
                def _safe(output_dir, device_ids):
                    # transient axon profiler failures (e.g. stop rc=-1)
                    # degrade to "no trace" instead of crashing the run
                    try:
                        cm = _raw(output_dir, device_ids)
                        cm.__enter__()
                    except Exception:
                        yield
                        return
                    try:
                        yield
                    finally:
                        try:
                            cm.__exit__(None, None, None)
                        except Exception:
                            pass

                hook = _safe

            mod = _t.ModuleType("antenv.axon_hooks")
            mod._hook = hook
            mod.set_axon_ntff_profile_hook = lambda h: setattr(mod, "_hook", h)
            mod.get_axon_ntff_profile_hook = lambda: mod._hook
            sys.modules["antenv.axon_hooks"] = mod
            antenv.axon_hooks = mod
        except Exception:
            pass
    from concourse import bass_utils as _bu

    if not getattr(_bu.upload_artifacts, "_safe_wrapped", False):
        _orig = _bu.upload_artifacts

        def _safe_upload(tmpdir):
            try:
                return _orig(tmpdir)
            except Exception:
                return f"file://{tmpdir}"

        _safe_upload._safe_wrapped = True
        _bu.upload_artifacts = _safe_upload

B, SEQ, DIM = 2048, 77, 768
VOCAB, EOT = 49408, 49407
INS = 16                       # appended rows: 8 da + 8 ca
TBL = VOCAB + INS
NCORES = 8
BPC = B // NCORES              # 256 batch rows per core
P = 128
NT = BPC // P                  # 2 partition tiles per core
MW = 2 * SEQ + 1               # meta width (indirect fallback)

MODE = "rawblock"
NSWQ = 4                       # SWDGE rings; gathers round-robined across them
TABLE_DT = "f16"               # "i8" (per-row int8 quant) measured SLOWER: SWDGE gen pays +210ns/inst on 768B descriptors
OUT_DT = TABLE_DT              # device output matches table dtype; host upcasts/dequants
GP_BUFS = 8
DMA_SCRATCH = 32768            # SWDGE descriptor-ring carveout bytes (split per ring)
CHUNKS = [11] * 7              # position chunking for the staging tiles
SC = 11

f32 = mybir.dt.float32
f16 = mybir.dt.float16
i32 = mybir.dt.int32
i8 = mybir.dt.int8
Alu = mybir.AluOpType
_DT = {"f32": f32, "f16": f16, "i8": i8}


def _build_hostidx() -> bass.Bass:
    tdt = _DT[TABLE_DT]
    odt = _DT[OUT_DT]
    nc = bacc.Bacc(
        "TRN2", dynamic_dma_scratch_size=DMA_SCRATCH, num_swdge_queues=NSWQ
    )
    idx_ext = nc.declare_dram_parameter("idx", [P, NT * SEQ], i32, isOutput=False)
    table_ext = nc.declare_dram_parameter("table", [TBL, DIM], tdt, isOutput=False)
    out_ext = nc.declare_dram_parameter("out", [BPC, SEQ * DIM], odt, isOutput=True)

    with tile.TileContext(nc) as tc:
        with (
            tc.tile_pool(name="ip", bufs=1) as ip,
            tc.tile_pool(name="gath", bufs=GP_BUFS) as gp,
        ):
            idx = ip.tile([P, NT * SEQ], i32, tag="idx")
            nc.sync.dma_start(out=idx[:], in_=idx_ext[:])
            qn = 0
            for t in range(NT):
                rows = slice(t * P, (t + 1) * P)
                s0 = 0
                for cl in CHUNKS:
                    g = gp.tile([P, cl, DIM], tdt, tag="g")
                    for j in range(cl):
                        c = t * SEQ + s0 + j
                        inst = nc.gpsimd.indirect_dma_start(
                            out=g[:, j, :],
                            out_offset=None,
                            in_=table_ext[:],
                            in_offset=bass.IndirectOffsetOnAxis(
                                ap=idx[:, c : c + 1], axis=0
                            ),
                        )
                        if NSWQ > 1:
                            q = qn % NSWQ
                            if q:
                                inst.ins.queue = f"qPoolDynamic{q}"
                            qn += 1
                    nc.sync.dma_start(
                        out=out_ext[rows, s0 * DIM : (s0 + cl) * DIM],
                        in_=g[:, :, :],
                    )
                    s0 += cl
    nc.finalize()
    return nc


RB_CHUNK = 11                  # rawblock: positions per staging chunk
RB_DEPTH = 8                   # rawblock: staging buffers (= sem pool size)


def _build_rawblock() -> bass.Bass:
    """Raw-block variant: manual semaphores instead of TileContext.

    The tile framework tracks every SWDGE gather with a rotating-8-sem
    wait+update pair, costing ~309ns of Pool sequencer time per gather on
    top of the ~1.1us SWDGE descriptor-gen. Here each gather carries only
    its completion inc (baked into the DMA descriptors); the gpsimd stream
    waits once per chunk (buffer reuse) and the SP stream waits once per
    chunk (gathers done) before the HWDGE write.
    """
    tdt = _DT[TABLE_DT]
    odt = _DT[OUT_DT]
    ncch = SEQ // RB_CHUNK
    nchunks = NT * ncch
    nsem = RB_DEPTH
    nc = bacc.Bacc(
        "TRN2", dynamic_dma_scratch_size=DMA_SCRATCH, num_swdge_queues=NSWQ
    )
    idx_ext = nc.declare_dram_parameter("idx", [P, NT * SEQ], i32, isOutput=False)
    table_ext = nc.declare_dram_parameter("table", [TBL, DIM], tdt, isOutput=False)
    out_ext = nc.declare_dram_parameter("out", [BPC, SEQ * DIM], odt, isOutput=True)

    from contextlib import ExitStack

    with (
        nc.Block() as block,
        nc.sbuf_tensor("idxs", [P, NT * SEQ], i32) as idxs,
        nc.sbuf_tensor("g", [P, RB_DEPTH, RB_CHUNK, DIM], tdt) as g,
        nc.semaphore("isem") as isem,
        ExitStack() as stack,
    ):
        gsems = [stack.enter_context(nc.semaphore(f"g{i}")) for i in range(nsem)]  # noqa: ANT232
        wsems = [stack.enter_context(nc.semaphore(f"w{i}")) for i in range(nsem)]  # noqa: ANT232

        @block.gpsimd
        def _(gp: bass.BassEngine):
            gp.wait_ge(isem, 16)
            qn = 0
            for c in range(nchunks):
                t, k = divmod(c, ncch)
                buf = c % RB_DEPTH
                if c >= RB_DEPTH:
                    # buffer reuse: write of chunk c-RB_DEPTH (same buf) done
                    gp.wait_ge(wsems[buf], 16 * (c // RB_DEPTH))
                for j in range(RB_CHUNK):
                    col = t * SEQ + k * RB_CHUNK + j
                    inst = gp.indirect_dma_start(
                        out=g[:, buf, j, :],
                        out_offset=None,
                        in_=table_ext[:],
                        in_offset=bass.IndirectOffsetOnAxis(
                            ap=idxs[:, col : col + 1], axis=0
                        ),
                    )
                    if NSWQ > 1:
                        q = qn % NSWQ
                        if q:
                            inst.ins.queue = f"qPoolDynamic{q}"
                        qn += 1
                    inst.then_inc(gsems[buf], 16)

        @block.sync
        def _(sy: bass.BassEngine):
            sy.dma_start(out=idxs[:], in_=idx_ext[:]).then_inc(isem, 16)
            for c in range(nchunks):
                t, k = divmod(c, ncch)
                buf = c % RB_DEPTH
                sy.wait_ge(gsems[buf], 16 * RB_CHUNK * (c // RB_DEPTH + 1))
                s0 = k * RB_CHUNK
                sy.dma_start(
                    out=out_ext[
                        slice(t * P, (t + 1) * P),
                        s0 * DIM : (s0 + RB_CHUNK) * DIM,
                    ],
                    in_=g[:, buf, :, :],
                ).then_inc(wsems[buf], 16)
            for i in range(nsem):
                nuses = (nchunks - 1 - i) // RB_DEPTH + 1
                sy.wait_ge(wsems[i], 16 * nuses)

    nc.finalize()
    return nc


def _build_indirect() -> bass.Bass:
    chunks = CHUNKS if CHUNKS is not None else [SC] * (SEQ // SC)
    assert sum(chunks) == SEQ
    tdt = f32 if TABLE_DT == "f32" else f16
    odt = f32 if OUT_DT == "f32" else f16
    nc = bacc.Bacc("TRN2", dynamic_dma_scratch_size=DMA_SCRATCH)
    meta_ext = nc.declare_dram_parameter("meta", [BPC, MW], f32, isOutput=False)
    table_ext = nc.declare_dram_parameter("table", [TBL, DIM], tdt, isOutput=False)
    out_ext = nc.declare_dram_parameter("out", [BPC, SEQ * DIM], odt, isOutput=True)

    with tile.TileContext(nc) as tc:
        with (
            tc.tile_pool(name="small", bufs=2) as sp,
            tc.tile_pool(name="gath", bufs=GP_BUFS) as gp,
        ):
            for t in range(NT):
                rows = slice(t * P, (t + 1) * P)

                meta = sp.tile([P, MW], f32, tag="meta")
                nc.sync.dma_start(out=meta[:], in_=meta_ext[rows, :])
                tok = meta[:, 0:SEQ]
                dyn = meta[:, SEQ : SEQ + 1]
                pos = meta[:, SEQ + 1 : SEQ + 1 + SEQ]

                iseq = sp.tile([P, SEQ], f32, tag="iseq")
                nc.vector.tensor_scalar(
                    out=iseq[:], in0=tok, scalar1=float(EOT), scalar2=None,
                    op0=Alu.is_equal,
                )
                pe = sp.tile([P, SEQ], f32, tag="pe")
                nc.vector.tensor_tensor(out=pe[:], in0=iseq[:], in1=pos, op=Alu.mult)
                e = sp.tile([P, 1], f32, tag="e")
                nc.vector.tensor_reduce(
                    out=e[:], in_=pe[:], axis=mybir.AxisListType.X, op=Alu.add
                )

                sh = sp.tile([P, 1], f32, tag="sh")
                nc.vector.tensor_scalar(
                    out=sh[:], in0=dyn, scalar1=8.0, scalar2=8.0,
                    op0=Alu.mult, op1=Alu.add,
                )
                eth = sp.tile([P, 1], f32, tag="eth")
                nc.vector.tensor_tensor(out=eth[:], in0=sh[:], in1=e[:], op=Alu.add)

                mid = sp.tile([P, SEQ], f32, tag="mid")
                nc.vector.tensor_scalar(
                    out=mid[:], in0=pos, scalar1=e[:], scalar2=float(VOCAB),
                    op0=Alu.subtract, op1=Alu.add,
                )

                m1 = sp.tile([P, SEQ], i32, tag="m1")
                nc.vector.tensor_scalar(
                    out=m1[:], in0=pos, scalar1=e[:], scalar2=None, op0=Alu.is_lt
                )
                m2 = sp.tile([P, SEQ], i32, tag="m2")
                nc.vector.tensor_scalar(
                    out=m2[:], in0=pos, scalar1=eth[:], scalar2=None, op0=Alu.is_lt
                )

                tm8 = sp.tile([P, SEQ], f32, tag="tm8")
                nc.vector.tensor_copy(out=tm8[:, 8:SEQ], in_=meta[:, 0 : SEQ - 8])
                nc.vector.tensor_copy(out=tm8[:, 0:8], in_=meta[:, 0:8])
                tm16 = sp.tile([P, SEQ], f32, tag="tm16")
                nc.vector.tensor_copy(out=tm16[:, 16:SEQ], in_=meta[:, 0 : SEQ - 16])
                nc.vector.tensor_copy(out=tm16[:, 0:16], in_=meta[:, 0:16])

                dd = sp.tile([P, SEQ], f32, tag="dd")
                nc.vector.tensor_tensor(out=dd[:], in0=tm16[:], in1=tm8[:], op=Alu.subtract)
                ddm = sp.tile([P, SEQ], f32, tag="ddm")
                nc.vector.tensor_scalar(
                    out=ddm[:], in0=dd[:], scalar1=dyn, scalar2=None, op0=Alu.mult
                )
                sel = sp.tile([P, SEQ], f32, tag="sel")
                nc.vector.tensor_tensor(out=sel[:], in0=tm8[:], in1=ddm[:], op=Alu.add)
                nc.vector.copy_predicated(out=sel[:], mask=m2[:], data=mid[:])
                nc.vector.copy_predicated(out=sel[:], mask=m1[:], data=tok)

                idx = sp.tile([P, SEQ], i32, tag="idx")
                nc.vector.tensor_copy(out=idx[:], in_=sel[:])

                s0 = 0
                for cl in chunks:
                    g = gp.tile([P, cl, DIM], tdt, tag="g")
                    for j in range(cl):
                        nc.gpsimd.indirect_dma_start(
                            out=g[:, j, :],
                            out_offset=None,
                            in_=table_ext[:],
                            in_offset=bass.IndirectOffsetOnAxis(
                                ap=idx[:, s0 + j : s0 + j + 1], axis=0
                            ),
                        )
                    nc.sync.dma_start(
                        out=out_ext[rows, s0 * DIM : (s0 + cl) * DIM],
                        in_=g[:, :, :],
                    )
                    s0 += cl
    nc.finalize()
    return nc


_cache: dict = {}


def _index_map(tokens: np.ndarray, dyn: np.ndarray) -> np.ndarray:
    """Full [B, SEQ] int32 index map into T = [emb; da; ca] (host side)."""
    eot = np.argmax(tokens == EOT, axis=1)
    shift = 8 + 8 * dyn.astype(np.int64)
    p = np.arange(SEQ, dtype=np.int64)[None, :]
    e = eot[:, None]
    sh = shift[:, None]
    src = np.clip(p - sh, 0, SEQ - 1)
    tok_shift = np.take_along_axis(tokens, src, axis=1)
    mid = VOCAB + (p - e)
    idx = np.where(p < e, tokens, np.where(p < e + sh, mid, tok_shift))
    return idx.astype(np.int32)


def _pack_meta(tokens_i32: np.ndarray, dyn_i32: np.ndarray) -> np.ndarray:
    meta = np.empty((B, MW), dtype=np.float32)
    meta[:, 0:SEQ] = tokens_i32
    meta[:, SEQ] = dyn_i32
    meta[:, SEQ + 1 :] = np.arange(SEQ, dtype=np.float32)[None, :]
    return meta


def kernel(**inputs) -> np.ndarray:
    _ensure_profiling_hooks()
    tokens = np.asarray(inputs["tokenized_text"]).astype(np.int64)
    dyn = np.asarray(inputs["dynamic_bools"]).astype(np.int64)
    emb = np.asarray(inputs["token_embedding"], dtype=np.float32)
    da = np.asarray(inputs["da_vectors"], dtype=np.float32)
    ca = np.asarray(inputs["ca_vectors"], dtype=np.float32)
    table = np.ascontiguousarray(np.concatenate([emb, da, ca], axis=0))
    scale = None
    if TABLE_DT == "f16":
        table = table.astype(np.float16)
    elif TABLE_DT == "i8":
        scale = np.maximum(np.abs(table).max(axis=1), 1e-12) / 127.0
        table = np.ascontiguousarray(
            np.round(table / scale[:, None]).astype(np.int8)
        )

    if MODE in ("hostidx", "rawblock"):
        idx = _index_map(tokens, dyn)
        in_maps = []
        for n in range(NCORES):
            ic = idx[n * BPC : (n + 1) * BPC]
            # device layout [128, NT*SEQ]: tile t occupies cols [t*SEQ, (t+1)*SEQ)
            packed = np.concatenate(
                [ic[t * P : (t + 1) * P] for t in range(NT)], axis=1
            )
            in_maps.append({"idx": np.ascontiguousarray(packed), "table": table})
        key = "nc_" + MODE
        if key not in _cache:
            _cache[key] = (
                _build_rawblock() if MODE == "rawblock" else _build_hostidx()
            )
        nc = _cache[key]
    else:
        meta = _pack_meta(tokens.astype(np.int32), dyn.astype(np.int32))
        in_maps = []
        for n in range(NCORES):
            rows = slice(n * BPC, (n + 1) * BPC)
            in_maps.append({"meta": meta[rows], "table": table})
        if "nc_i" not in _cache:
            _cache["nc_i"] = _build_indirect()
        nc = _cache["nc_i"]

    res = run_bass_kernel_spmd(nc, in_maps, core_ids=list(range(NCORES)))
    _cache["last_results"] = res
    out = np.concatenate(
        [res.results[i]["out"].reshape(BPC, SEQ, DIM) for i in range(NCORES)],
        axis=0,
    )
    if OUT_DT == "i8":
        out = out.astype(np.float32) * scale[idx][:, :, None]
    elif out.dtype != np.float32:
        out = out.astype(np.float32)
    return out
